# revision 2
# baseline (speedup 1.0000x reference)
"""Trainium2 Bass kernel for the BN + 1x1-conv self-attention block.

Reference computation (per batch item, c=256 channels, n=4096 tokens):
  BN(x) over (b,h,w) -> qkv = W_qkv @ xn -> attention -> W_out proj -> +x

Sharding: 8 cores = 4 batch items x 2 query-halves, fully collective-free.
Each core:
  - receives ALL of x in bf16, [c, 16384], columns ordered so its own
    2048 query positions come first, then the rest of its item, then the
    other 3 items (attention is permutation-invariant in the key axis;
    BN statistics are permutation-invariant over all positions).
  - computes exact global BN statistics locally: bn_stats over all
    16384 columns per channel (replicated across cores; this trades
    ~8 MB of extra HBM reads for zero cross-core communication -- the
    AllReduce/AllGather path through the emulated runtime costs
    milliseconds per launch).
  - folds BN into the QKV conv: W' = W * s_c (per input channel),
    b' = b_qkv + W @ t. The V-channel shift contributes a constant
    per-channel offset to the attention output (softmax weights sum
    to 1), applied post-attention as a per-partition bias.
  - attention is computed in transposed form: S^T[k,q] tiles flow
    scores -> exp -> (A^T V -> out^T[c,q]). The softmax denominator is
    accumulated on DVE (exp-tile adds) and reduced over partitions by a
    single ones-vector matmul per query chunk; normalization is applied
    at the end (no max-subtraction needed: logits are ~N(0,1) after BN).
    The k-tile loop is software-pipelined: AV(kt-1) runs on PE while
    ACT computes exp(kt), so PE has no exp-latency bubble.
  - the residual + output use an fp32 copy of the core's 2048 query
    columns (x itself is consumed in bf16 only by matmuls/statistics).

All heavy matmuls use bf16 operands. Logits need no max-subtraction:
they are ~N(0,1) after BN, so exp() is safe in fp32.
"""
import sys

sys.path.append("/opt/trn_rl_repo")

import numpy as np
from contextlib import ExitStack

import concourse.bass as bass
import concourse.tile as tile
from concourse import bacc, mybir
from concourse import bass_utils

F32 = mybir.dt.float32
BF16 = mybir.dt.bfloat16
F32R = mybir.dt.float32r
AF = mybir.ActivationFunctionType
ALU = mybir.AluOpType

B, C, H, W = 4, 256, 64, 64
NPOS = H * W          # 4096 positions per item
NQ = NPOS // 2        # 2048 query positions per core
NALL = B * NPOS       # 16384 positions for BN statistics
N_CORES = 8
CT = C // 128         # 2 channel partition-tiles
OT = 3 * C // 128     # 6 qkv output tiles
EPS = 1e-5
SCALE = C ** (-0.5)   # 1/16
N_WARMUP_MM = 140
CHUNK = 2048          # DMA chunk (columns) for the x stream
ATTN_DT = mybir.dt.bfloat16  # dtype for Q/K/V/attn-weight matmul operands
MM_DT = mybir.dt.bfloat16    # dtype for x / conv-weight matmul operands


def _build(n_reps: int = 1, n_qc: int = 4):
    nc = bacc.Bacc("TRN2", target_bir_lowering=False, debug=False)

    xb = nc.dram_tensor("xb", [C, NALL], BF16, kind="ExternalInput")
    xq32 = nc.dram_tensor("xq32", [C, NQ], F32, kind="ExternalInput")
    w_qkv_t = nc.dram_tensor("w_qkv_t", [C, 3 * C], F32, kind="ExternalInput")
    w_out_t = nc.dram_tensor("w_out_t", [C, C], F32, kind="ExternalInput")
    b_qkv = nc.dram_tensor("b_qkv", [3 * C, 1], F32, kind="ExternalInput")
    b_out = nc.dram_tensor("b_out", [C, 1], F32, kind="ExternalInput")
    gamma = nc.dram_tensor("gamma", [C, 1], F32, kind="ExternalInput")
    beta = nc.dram_tensor("beta", [C, 1], F32, kind="ExternalInput")
    out_d = nc.dram_tensor("out", [C, NQ], F32, kind="ExternalOutput")

    NCHUNK = NALL // CHUNK          # 8 chunks per channel-tile
    NOWN = NPOS // CHUNK            # first 2 chunks hold the own item
    GPC = CHUNK // 512              # bn_stats groups per chunk

    with tile.TileContext(nc) as tc:
        with ExitStack() as ctx:
            big = ctx.enter_context(tc.tile_pool(name="big", bufs=1))
            strm = ctx.enter_context(tc.tile_pool(name="strm", bufs=6))
            stage = ctx.enter_context(tc.tile_pool(name="stage", bufs=2))
            vec = ctx.enter_context(tc.tile_pool(name="vec", bufs=1))
            expp = ctx.enter_context(tc.tile_pool(name="expp", bufs=8))
            attnp = ctx.enter_context(tc.tile_pool(name="attnp", bufs=4))
            outp = ctx.enter_context(tc.tile_pool(name="outp", bufs=4))
            dnp = ctx.enter_context(tc.tile_pool(name="dnp", bufs=2))
            ps_s = ctx.enter_context(tc.tile_pool(name="ps_s", bufs=2, space="PSUM"))
            ps_av = ctx.enter_context(tc.tile_pool(name="ps_av", bufs=4, space="PSUM"))
            ps_x = ctx.enter_context(tc.tile_pool(name="ps_x", bufs=2, space="PSUM"))

            for _rep in range(n_reps):
                # ---- weights first on the sync queue (gate the BN fold) ----
                w_f32 = []
                for ct in range(CT):
                    wt = big.tile([128, 3 * C], F32, tag=f"w_f32_{ct}", name=f"w_f32_{ct}")
                    nc.sync.dma_start(wt[:], w_qkv_t[128 * ct:128 * (ct + 1), :])
                    w_f32.append(wt)
                wout_r = []
                for ct in range(CT):
                    ws = stage.tile([128, C], F32, tag="wout_stage", name="wout_stage")
                    nc.sync.dma_start(ws[:], w_out_t[128 * ct:128 * (ct + 1), :])
                    wr = big.tile([128, C], MM_DT, tag=f"wout_r_{ct}", name=f"wout_r_{ct}")
                    nc.vector.tensor_copy(wr[:], ws[:])
                    wout_r.append(wr)

                # ---- x stream: own item persistent, rest through a ring ----
                # ct0 on the sync queue, ct1 on the scalar queue.
                xo = []
                stats_t = []
                for ct in range(CT):
                    eng = nc.sync if ct == 0 else nc.scalar
                    xot = big.tile([128, NPOS], BF16, tag=f"xo_{ct}", name=f"xo_{ct}")
                    st = vec.tile([128, NALL // 512, 6], F32, tag=f"bnst_{ct}",
                                  name=f"bnst_{ct}")
                    for j in range(NOWN):
                        sl = slice(CHUNK * j, CHUNK * (j + 1))
                        eng.dma_start(xot[:, sl], xb[128 * ct:128 * (ct + 1), sl])
                        xg = xot[:, sl].rearrange("p (n f) -> p n f", f=512)
                        for i in range(GPC):
                            nc.vector.bn_stats(out=st[:, GPC * j + i, :], in_=xg[:, i, :])
                    xo.append(xot)
                    stats_t.append(st)
                for ct in range(CT):
                    eng = nc.sync if ct == 0 else nc.scalar
                    for j in range(NOWN, NCHUNK):
                        sl = slice(CHUNK * j, CHUNK * (j + 1))
                        xs = strm.tile([128, CHUNK], BF16, tag="xstrm",
                                       name=f"xs_{ct}_{j}")
                        eng.dma_start(xs[:], xb[128 * ct:128 * (ct + 1), sl])
                        xg = xs[:].rearrange("p (n f) -> p n f", f=512)
                        for i in range(GPC):
                            nc.vector.bn_stats(out=stats_t[ct][:, GPC * j + i, :],
                                               in_=xg[:, i, :])

                # fp32 query columns for the residual (needed only at the end)
                xq = []
                for ct in range(CT):
                    xqt = big.tile([128, NQ], F32, tag=f"xq_{ct}", name=f"xq_{ct}")
                    nc.scalar.dma_start(xqt[:], xq32[128 * ct:128 * (ct + 1), :])
                    xq.append(xqt)

                # small per-channel vectors on the gpsimd (SWDGE) queue
                bq_col, bo_col = [], []
                for ot in range(OT):
                    t = vec.tile([128, 1], F32, tag=f"bq_col_{ot}", name=f"bq_col_{ot}")
                    nc.gpsimd.dma_start(t[:], b_qkv[128 * ot:128 * (ot + 1), :])
                    bq_col.append(t)
                for ct in range(CT):
                    t = vec.tile([128, 1], F32, tag=f"bo_{ct}", name=f"bo_{ct}")
                    nc.gpsimd.dma_start(t[:], b_out[128 * ct:128 * (ct + 1), :])
                    bo_col.append(t)
                # gamma/beta as [128, 2] (channel-tile on the free dim)
                ga2 = vec.tile([128, CT], F32, tag="ga2")
                nc.gpsimd.dma_start(ga2[:], gamma[:].rearrange("(c p) one -> p (c one)", p=128))
                be2 = vec.tile([128, CT], F32, tag="be2")
                nc.gpsimd.dma_start(be2[:], beta[:].rearrange("(c p) one -> p (c one)", p=128))

                # PE warm-up while the x stream is in flight (first rep only:
                # in steady state the previous rep keeps the PE array warm)
                if _rep == 0:
                    warm_f = stage.tile([128, 512], F32, tag="warm_f", name="warm_f", bufs=1)
                    nc.vector.memset(warm_f[:], 0.5)
                    warm_r = stage.tile([128, 512], MM_DT, tag="warm_r", name="warm_r", bufs=1)
                    nc.vector.tensor_copy(warm_r[:], warm_f[:])
                    for wi in range(N_WARMUP_MM):
                        pw = ps_s.tile([128, 512], F32, tag="s", name=f"warm_{wi}")
                        nc.tensor.matmul(
                            pw[:], warm_r[:, 0:128], warm_r[:],
                            start=True, stop=True,
                        )

                eps_col = vec.tile([128, 1], F32, tag="eps_col")
                nc.vector.memset(eps_col[:], EPS)
                # dummy Sqrt: pulls the ACT table load off the stats chain
                sq_warm = vec.tile([128, 1], F32, tag="sq_warm")
                nc.scalar.activation(sq_warm[:], eps_col[:], AF.Sqrt)
                ones_col_f = vec.tile([128, 1], F32, tag="ones_col_f")
                nc.vector.memset(ones_col_f[:], 1.0)
                ones_row_f = vec.tile([1, 128], F32, tag="ones_row_f")
                nc.vector.memset(ones_row_f[:], 1.0)
                ones_row_r = vec.tile([1, 128], F32R, tag="ones_row_r")
                nc.vector.tensor_copy(ones_row_r[:], ones_row_f[:])

                # ---- exact global stats -> s (scale) and t (shift) ----
                s_col, t_col = [], []
                for ct in range(CT):
                    mv = vec.tile([128, 2], F32, tag=f"mv_{ct}", name=f"mv_{ct}")
                    nc.vector.bn_aggr(out=mv[:], in_=stats_t[ct][:])
                    sd = vec.tile([128, 1], F32, tag=f"sd_{ct}", name=f"sd_{ct}")
                    nc.scalar.activation(sd[:], mv[:, 1:2], AF.Sqrt, bias=eps_col[:])
                    s1 = vec.tile([128, 1], F32, tag=f"s1_{ct}", name=f"s1_{ct}")
                    nc.vector.reciprocal(s1[:], sd[:])
                    nc.vector.tensor_mul(s1[:], s1[:], ga2[:, ct:ct + 1])
                    t1 = vec.tile([128, 1], F32, tag=f"t1_{ct}", name=f"t1_{ct}")
                    nc.vector.tensor_mul(t1[:], mv[:, 0:1], s1[:])
                    nc.vector.tensor_tensor(out=t1[:], in0=be2[:, ct:ct + 1], in1=t1[:],
                                            op=ALU.subtract)
                    s_col.append(s1)
                    t_col.append(t1)

                # ---- fold BN into weights ----
                bq_fold = []
                for ot in range(OT):
                    pbq = ps_x.tile([128, 1], F32, tag="x", name=f"pbq_{ot}")
                    for ct in range(CT):
                        nc.tensor.matmul(
                            pbq[:],
                            w_f32[ct][:, 128 * ot:128 * (ot + 1)],
                            t_col[ct][:],
                            start=(ct == 0), stop=(ct == CT - 1),
                        )
                    bqf = vec.tile([128, 1], F32, tag=f"bqf_{ot}", name=f"bqf_{ot}")
                    nc.vector.tensor_add(bqf[:], pbq[:], bq_col[ot][:])
                    bq_fold.append(bqf)
                wqkv_r = []
                for ct in range(CT):
                    wr = big.tile([128, 3 * C], MM_DT, tag=f"wqkv_r_{ct}", name=f"wqkv_r_{ct}")
                    nc.vector.tensor_scalar_mul(wr[:], w_f32[ct][:], s_col[ct][:])
                    wqkv_r.append(wr)

                # ---- QKV projections ----
                qt_r = [big.tile([128, NQ], ATTN_DT, tag=f"qt_{ct}", name=f"qt_{ct}") for ct in range(CT)]
                k_r = [big.tile([128, NPOS], ATTN_DT, tag=f"k_{ct}", name=f"k_{ct}") for ct in range(CT)]
                qkv_i = 0
                for ot in range(4):  # o-tiles 0,1 -> Q ; 2,3 -> K
                    is_q = ot < 2
                    npc = 4 if is_q else 8
                    for pc in range(npc):
                        pool = ps_s if qkv_i % 2 == 0 else ps_x
                        tg = "s" if qkv_i % 2 == 0 else "x"
                        qkv_i += 1
                        ps = pool.tile([128, 512], F32, tag=tg, name=f"qkv_{ot}_{pc}")
                        for ct in range(CT):
                            nc.tensor.matmul(
                                ps[:],
                                wqkv_r[ct][:, 128 * ot:128 * (ot + 1)],
                                xo[ct][:, 512 * pc:512 * (pc + 1)],
                                start=(ct == 0), stop=(ct == CT - 1),
                            )
                        if is_q:
                            dest = qt_r[ot][:, 512 * pc:512 * (pc + 1)]
                        else:
                            dest = k_r[ot - 2][:, 512 * pc:512 * (pc + 1)]
                        nc.vector.tensor_scalar_add(dest, ps[:], bq_fold[ot][:])
                v_r = big.tile([128, 32 * C], ATTN_DT, tag="v_r")
                for pt in range(32):
                    psv = ps_x.tile([128, C], F32, tag="x", name=f"v_{pt}")
                    for ct in range(CT):
                        nc.tensor.matmul(
                            psv[:],
                            xo[ct][:, 128 * pt:128 * (pt + 1)],
                            wqkv_r[ct][:, 2 * C:3 * C],
                            start=(ct == 0), stop=(ct == CT - 1),
                        )
                    nc.vector.tensor_copy(v_r[:, C * pt:C * (pt + 1)], psv[:])

                # ---- attention, software-pipelined over 32 k-tiles ----
                def scores(qs, kt, qc):
                    ss = ps_s.tile([128, 512], F32, tag="s", name=f"ss_{qc}_{kt}")
                    for ct in range(CT):
                        nc.tensor.matmul(
                            ss[:],
                            k_r[ct][:, 128 * kt:128 * (kt + 1)],
                            qt_r[ct][:, qs],
                            start=(ct == 0), stop=(ct == CT - 1),
                        )
                    ex = expp.tile([128, 512], ATTN_DT, tag="ex", name=f"ex_{qc}_{kt}")
                    nc.scalar.activation(ex[:], ss[:], AF.Exp, scale=SCALE)
                    return ex

                for qc in range(n_qc):
                    qs = slice(512 * qc, 512 * (qc + 1))
                    av = [ps_av.tile([128, 512], F32, tag="av", name=f"av_{qc}_{i}")
                          for i in range(CT)]
                    # softmax denominator accumulates on DVE (frees PE time)
                    acc = dnp.tile([128, 512], F32, tag="dnacc", name=f"dnacc_{qc}")

                    def av_step(ex, kt, av=av):
                        for ct in range(CT):
                            nc.tensor.matmul(
                                av[ct][:],
                                v_r[:, C * kt + 128 * ct:C * kt + 128 * (ct + 1)],
                                ex[:],
                                start=(kt == 0), stop=(kt == 31),
                            )

                    prev_ex = scores(qs, 0, qc)
                    for kt in range(1, 32):
                        ex = scores(qs, kt, qc)
                        av_step(prev_ex, kt - 1)
                        if kt == 1:
                            nc.vector.tensor_tensor(
                                out=acc[:], in0=prev_ex[:], in1=ex[:], op=ALU.add)
                        else:
                            nc.vector.tensor_add(acc[:], acc[:], ex[:])
                        prev_ex = ex
                    av_step(prev_ex, 31)
                    # partition reduction of the accumulated exp sums: one
                    # fp32 ones-matmul (DVE lanes cannot cross partitions)
                    dnps = ps_x.tile([1, 512], F32, tag="x", name=f"dnps_{qc}")
                    nc.tensor.matmul(dnps[:], ones_col_f[:], acc[:], start=True, stop=True)

                    # normalize: attn^T[c,q] = av * (1/dn) + bv'
                    rec = vec.tile([1, 512], F32, tag="rec", name=f"rec_{qc}")
                    nc.vector.reciprocal(rec[:], dnps[:])
                    rec_r = vec.tile([1, 512], F32R, tag="rec_r", name=f"recr_{qc}")
                    nc.vector.tensor_copy(rec_r[:], rec[:])
                    bc = ps_x.tile([128, 512], F32, tag="x", name=f"bc_{qc}")
                    nc.tensor.matmul(bc[:], ones_row_r[:], rec_r[:], start=True, stop=True)
                    bc_sb = attnp.tile([128, 512], F32, tag="bc_sb", name=f"bcsb_{qc}")
                    nc.scalar.mul(bc_sb[:], bc[:], 1.0)
                    at_sb = []
                    for ct in range(CT):
                        at = attnp.tile([128, 512], MM_DT, tag=f"at_{ct}", name=f"at_{qc}_{ct}")
                        nc.vector.tensor_tensor(out=at[:], in0=av[ct][:], in1=bc_sb[:], op=ALU.mult)
                        nc.vector.tensor_scalar_add(at[:], at[:], bq_fold[4 + ct][:])
                        at_sb.append(at)
                    # output projection + bias + residual
                    for ot in range(CT):
                        po = ps_x.tile([128, 512], F32, tag="x", name=f"po_{qc}_{ot}")
                        for ct in range(CT):
                            nc.tensor.matmul(
                                po[:],
                                wout_r[ct][:, 128 * ot:128 * (ot + 1)],
                                at_sb[ct][:],
                                start=(ct == 0), stop=(ct == CT - 1),
                            )
                        fin = outp.tile([128, 512], F32, tag="fin", name=f"fin_{qc}_{ot}")
                        nc.vector.tensor_scalar_add(fin[:], po[:], bo_col[ot][:])
                        nc.vector.tensor_tensor(out=fin[:], in0=fin[:], in1=xq[ot][:, qs], op=ALU.add)
                        nc.sync.dma_start(out_d[128 * ot:128 * (ot + 1), qs], fin[:])

    nc.finalize()
    return nc


_NC_CACHE = None


def _get_nc(n_reps: int = 1):
    global _NC_CACHE
    if _NC_CACHE is None:
        _NC_CACHE = _build(n_reps)
    return _NC_CACHE


def make_in_maps(inputs):
    """Per-core input staging: layout/dtype only (no math)."""
    x = np.asarray(inputs["x"], np.float32)
    W_qkv = np.asarray(inputs["W_qkv"], np.float32)
    W_out = np.asarray(inputs["W_out"], np.float32)
    bf16 = mybir.dt.np(BF16)

    w_qkv_t = np.ascontiguousarray(W_qkv.T)          # [256, 768]
    w_out_t = np.ascontiguousarray(W_out.T)          # [256, 256]
    bq2 = np.asarray(inputs["b_qkv"], np.float32).reshape(3 * C, 1)
    bo2 = np.asarray(inputs["b_out"], np.float32).reshape(C, 1)
    ga2 = np.asarray(inputs["gamma"], np.float32).reshape(C, 1)
    be2 = np.asarray(inputs["beta"], np.float32).reshape(C, 1)

    xf = x.reshape(B, C, NPOS)
    in_maps = []
    for core in range(N_CORES):
        item, half = divmod(core, 2)
        xi = xf[item]
        if half == 0:
            own = xi
        else:
            own = np.concatenate([xi[:, NQ:], xi[:, :NQ]], axis=1)
        others = [xf[j] for j in range(B) if j != item]
        xb_np = np.ascontiguousarray(
            np.concatenate([own] + others, axis=1)).astype(bf16)
        in_maps.append({
            "xb": xb_np,
            "xq32": np.ascontiguousarray(own[:, :NQ]),
            "w_qkv_t": w_qkv_t,
            "w_out_t": w_out_t,
            "b_qkv": bq2,
            "b_out": bo2,
            "gamma": ga2,
            "beta": be2,
        })
    return in_maps


def kernel(x, W_qkv, b_qkv, W_out, b_out, gamma, beta):
    nc = _get_nc()
    in_maps = make_in_maps({
        "x": x, "W_qkv": W_qkv, "b_qkv": b_qkv, "W_out": W_out,
        "b_out": b_out, "gamma": gamma, "beta": beta,
    })
    res = bass_utils.run_bass_kernel_spmd(nc, in_maps, core_ids=list(range(N_CORES)))

    out = np.empty((B, C, NPOS), dtype=np.float32)
    for core in range(N_CORES):
        item, half = divmod(core, 2)
        out[item][:, NQ * half:NQ * (half + 1)] = res.results[core]["out"]
    return out.reshape(B, C, H, W)


# revision 7
# speedup vs baseline: 1.2470x; 1.2470x over previous
"""Trainium2 Bass kernel for the BN + 1x1-conv self-attention block.

Known-good bf16 fallback (measured: rel err 6.87e-4, HW marginal body
196293 ns): collective-free replicated-stats version with bf16 attention.

Sharding: 8 cores = 4 batch items x 2 query-halves, fully collective-free.
Each core receives ALL of x in bf16 (columns ordered: own 2048 queries,
own item rest, other 3 items), computes exact global BN statistics
locally via bn_stats/bn_aggr, folds BN into the QKV conv, and runs
transposed-form attention in bf16.
"""
import sys

sys.path.append("/opt/trn_rl_repo")

import numpy as np
from contextlib import ExitStack

import concourse.bass as bass
import concourse.tile as tile
from concourse import bacc, mybir
from concourse import bass_utils

F32 = mybir.dt.float32
BF16 = mybir.dt.bfloat16
F32R = mybir.dt.float32r
AF = mybir.ActivationFunctionType
ALU = mybir.AluOpType

B, C, H, W = 4, 256, 64, 64
NPOS = H * W          # 4096 positions per item
NQ = NPOS // 2        # 2048 query positions per core
NALL = B * NPOS       # 16384 positions for BN statistics
N_CORES = 8
CT = C // 128         # 2 channel partition-tiles
OT = 3 * C // 128     # 6 qkv output tiles
EPS = 1e-5
SCALE = C ** (-0.5)   # 1/16
N_WARMUP_MM = 140
CHUNK = 2048          # DMA chunk (columns) for the x stream
ATTN_DT = mybir.dt.bfloat16  # dtype for Q/K/V/attn-weight matmul operands
MM_DT = mybir.dt.bfloat16    # dtype for x / conv-weight matmul operands


def _build(n_reps: int = 1, n_qc: int = 4):
    nc = bacc.Bacc("TRN2", target_bir_lowering=False, debug=False)

    xb = nc.dram_tensor("xb", [C, NALL], BF16, kind="ExternalInput")
    xq32 = nc.dram_tensor("xq32", [C, NQ], F32, kind="ExternalInput")
    w_qkv_t = nc.dram_tensor("w_qkv_t", [C, 3 * C], F32, kind="ExternalInput")
    w_out_t = nc.dram_tensor("w_out_t", [C, C], F32, kind="ExternalInput")
    b_qkv = nc.dram_tensor("b_qkv", [3 * C, 1], F32, kind="ExternalInput")
    b_out = nc.dram_tensor("b_out", [C, 1], F32, kind="ExternalInput")
    gamma = nc.dram_tensor("gamma", [C, 1], F32, kind="ExternalInput")
    beta = nc.dram_tensor("beta", [C, 1], F32, kind="ExternalInput")
    out_d = nc.dram_tensor("out", [C, NQ], F32, kind="ExternalOutput")

    NCHUNK = NALL // CHUNK          # 8 chunks per channel-tile
    NOWN = NPOS // CHUNK            # first 2 chunks hold the own item
    GPC = CHUNK // 512              # bn_stats groups per chunk

    with tile.TileContext(nc) as tc:
        with ExitStack() as ctx:
            big = ctx.enter_context(tc.tile_pool(name="big", bufs=1))
            strm = ctx.enter_context(tc.tile_pool(name="strm", bufs=6))
            stage = ctx.enter_context(tc.tile_pool(name="stage", bufs=2))
            vec = ctx.enter_context(tc.tile_pool(name="vec", bufs=1))
            expp = ctx.enter_context(tc.tile_pool(name="expp", bufs=8))
            attnp = ctx.enter_context(tc.tile_pool(name="attnp", bufs=4))
            outp = ctx.enter_context(tc.tile_pool(name="outp", bufs=4))
            dnp = ctx.enter_context(tc.tile_pool(name="dnp", bufs=2))
            ps_s = ctx.enter_context(tc.tile_pool(name="ps_s", bufs=2, space="PSUM"))
            ps_av = ctx.enter_context(tc.tile_pool(name="ps_av", bufs=4, space="PSUM"))
            ps_x = ctx.enter_context(tc.tile_pool(name="ps_x", bufs=2, space="PSUM"))

            for _rep in range(n_reps):
                # ---- weights first on the sync queue (gate the BN fold) ----
                w_f32 = []
                for ct in range(CT):
                    wt = big.tile([128, 3 * C], F32, tag=f"w_f32_{ct}", name=f"w_f32_{ct}")
                    nc.sync.dma_start(wt[:], w_qkv_t[128 * ct:128 * (ct + 1), :])
                    w_f32.append(wt)
                wout_r = []
                for ct in range(CT):
                    ws = stage.tile([128, C], F32, tag="wout_stage", name="wout_stage")
                    nc.sync.dma_start(ws[:], w_out_t[128 * ct:128 * (ct + 1), :])
                    wr = big.tile([128, C], MM_DT, tag=f"wout_r_{ct}", name=f"wout_r_{ct}")
                    nc.vector.tensor_copy(wr[:], ws[:])
                    wout_r.append(wr)

                # ---- x stream: own item persistent, rest through a ring ----
                xo = []
                stats_t = []
                for ct in range(CT):
                    eng = nc.sync if ct == 0 else nc.scalar
                    xot = big.tile([128, NPOS], BF16, tag=f"xo_{ct}", name=f"xo_{ct}")
                    st = vec.tile([128, NALL // 512, 6], F32, tag=f"bnst_{ct}",
                                  name=f"bnst_{ct}")
                    for j in range(NOWN):
                        sl = slice(CHUNK * j, CHUNK * (j + 1))
                        eng.dma_start(xot[:, sl], xb[128 * ct:128 * (ct + 1), sl])
                        xg = xot[:, sl].rearrange("p (n f) -> p n f", f=512)
                        for i in range(GPC):
                            nc.vector.bn_stats(out=st[:, GPC * j + i, :], in_=xg[:, i, :])
                    xo.append(xot)
                    stats_t.append(st)
                for ct in range(CT):
                    eng = nc.sync if ct == 0 else nc.scalar
                    for j in range(NOWN, NCHUNK):
                        sl = slice(CHUNK * j, CHUNK * (j + 1))
                        xs = strm.tile([128, CHUNK], BF16, tag="xstrm",
                                       name=f"xs_{ct}_{j}")
                        eng.dma_start(xs[:], xb[128 * ct:128 * (ct + 1), sl])
                        xg = xs[:].rearrange("p (n f) -> p n f", f=512)
                        for i in range(GPC):
                            nc.vector.bn_stats(out=stats_t[ct][:, GPC * j + i, :],
                                               in_=xg[:, i, :])

                # fp32 query columns for the residual (needed only at the end)
                xq = []
                for ct in range(CT):
                    xqt = big.tile([128, NQ], F32, tag=f"xq_{ct}", name=f"xq_{ct}")
                    nc.scalar.dma_start(xqt[:], xq32[128 * ct:128 * (ct + 1), :])
                    xq.append(xqt)

                # small per-channel vectors on the gpsimd (SWDGE) queue
                bq_col, bo_col = [], []
                for ot in range(OT):
                    t = vec.tile([128, 1], F32, tag=f"bq_col_{ot}", name=f"bq_col_{ot}")
                    nc.gpsimd.dma_start(t[:], b_qkv[128 * ot:128 * (ot + 1), :])
                    bq_col.append(t)
                for ct in range(CT):
                    t = vec.tile([128, 1], F32, tag=f"bo_{ct}", name=f"bo_{ct}")
                    nc.gpsimd.dma_start(t[:], b_out[128 * ct:128 * (ct + 1), :])
                    bo_col.append(t)
                # gamma/beta as [128, 2] (channel-tile on the free dim)
                ga2 = vec.tile([128, CT], F32, tag="ga2")
                nc.gpsimd.dma_start(ga2[:], gamma[:].rearrange("(c p) one -> p (c one)", p=128))
                be2 = vec.tile([128, CT], F32, tag="be2")
                nc.gpsimd.dma_start(be2[:], beta[:].rearrange("(c p) one -> p (c one)", p=128))

                # PE warm-up while the x stream is in flight (first rep only)
                if _rep == 0:
                    warm_f = stage.tile([128, 512], F32, tag="warm_f", name="warm_f", bufs=1)
                    nc.vector.memset(warm_f[:], 0.5)
                    warm_r = stage.tile([128, 512], MM_DT, tag="warm_r", name="warm_r", bufs=1)
                    nc.vector.tensor_copy(warm_r[:], warm_f[:])
                    for wi in range(N_WARMUP_MM):
                        pw = ps_s.tile([128, 512], F32, tag="s", name=f"warm_{wi}")
                        nc.tensor.matmul(
                            pw[:], warm_r[:, 0:128], warm_r[:],
                            start=True, stop=True,
                        )

                eps_col = vec.tile([128, 1], F32, tag="eps_col")
                nc.vector.memset(eps_col[:], EPS)
                # dummy Sqrt: pulls the ACT table load off the stats chain
                sq_warm = vec.tile([128, 1], F32, tag="sq_warm")
                nc.scalar.activation(sq_warm[:], eps_col[:], AF.Sqrt)
                ones_col_f = vec.tile([128, 1], F32, tag="ones_col_f")
                nc.vector.memset(ones_col_f[:], 1.0)
                ones_row_f = vec.tile([1, 128], F32, tag="ones_row_f")
                nc.vector.memset(ones_row_f[:], 1.0)
                ones_row_r = vec.tile([1, 128], F32R, tag="ones_row_r")
                nc.vector.tensor_copy(ones_row_r[:], ones_row_f[:])

                # ---- exact global stats -> s (scale) and t (shift) ----
                s_col, t_col = [], []
                for ct in range(CT):
                    mv = vec.tile([128, 2], F32, tag=f"mv_{ct}", name=f"mv_{ct}")
                    nc.vector.bn_aggr(out=mv[:], in_=stats_t[ct][:])
                    sd = vec.tile([128, 1], F32, tag=f"sd_{ct}", name=f"sd_{ct}")
                    nc.scalar.activation(sd[:], mv[:, 1:2], AF.Sqrt, bias=eps_col[:])
                    s1 = vec.tile([128, 1], F32, tag=f"s1_{ct}", name=f"s1_{ct}")
                    nc.vector.reciprocal(s1[:], sd[:])
                    nc.vector.tensor_mul(s1[:], s1[:], ga2[:, ct:ct + 1])
                    t1 = vec.tile([128, 1], F32, tag=f"t1_{ct}", name=f"t1_{ct}")
                    nc.vector.tensor_mul(t1[:], mv[:, 0:1], s1[:])
                    nc.vector.tensor_tensor(out=t1[:], in0=be2[:, ct:ct + 1], in1=t1[:],
                                            op=ALU.subtract)
                    s_col.append(s1)
                    t_col.append(t1)

                # ---- fold BN into weights ----
                bq_fold = []
                for ot in range(OT):
                    pbq = ps_x.tile([128, 1], F32, tag="x", name=f"pbq_{ot}")
                    for ct in range(CT):
                        nc.tensor.matmul(
                            pbq[:],
                            w_f32[ct][:, 128 * ot:128 * (ot + 1)],
                            t_col[ct][:],
                            start=(ct == 0), stop=(ct == CT - 1),
                        )
                    bqf = vec.tile([128, 1], F32, tag=f"bqf_{ot}", name=f"bqf_{ot}")
                    nc.vector.tensor_add(bqf[:], pbq[:], bq_col[ot][:])
                    bq_fold.append(bqf)
                wqkv_r = []
                for ct in range(CT):
                    wr = big.tile([128, 3 * C], MM_DT, tag=f"wqkv_r_{ct}", name=f"wqkv_r_{ct}")
                    nc.vector.tensor_scalar_mul(wr[:], w_f32[ct][:], s_col[ct][:])
                    wqkv_r.append(wr)

                # ---- QKV projections ----
                qt_r = [big.tile([128, NQ], ATTN_DT, tag=f"qt_{ct}", name=f"qt_{ct}") for ct in range(CT)]
                k_r = [big.tile([128, NPOS], ATTN_DT, tag=f"k_{ct}", name=f"k_{ct}") for ct in range(CT)]
                qkv_i = 0
                for ot in range(4):  # o-tiles 0,1 -> Q ; 2,3 -> K
                    is_q = ot < 2
                    npc = 4 if is_q else 8
                    for pc in range(npc):
                        pool = ps_s if qkv_i % 2 == 0 else ps_x
                        tg = "s" if qkv_i % 2 == 0 else "x"
                        qkv_i += 1
                        ps = pool.tile([128, 512], F32, tag=tg, name=f"qkv_{ot}_{pc}")
                        for ct in range(CT):
                            nc.tensor.matmul(
                                ps[:],
                                wqkv_r[ct][:, 128 * ot:128 * (ot + 1)],
                                xo[ct][:, 512 * pc:512 * (pc + 1)],
                                start=(ct == 0), stop=(ct == CT - 1),
                            )
                        if is_q:
                            dest = qt_r[ot][:, 512 * pc:512 * (pc + 1)]
                        else:
                            dest = k_r[ot - 2][:, 512 * pc:512 * (pc + 1)]
                        nc.vector.tensor_scalar_add(dest, ps[:], bq_fold[ot][:])
                v_r = big.tile([128, 32 * C], ATTN_DT, tag="v_r")
                for pt in range(32):
                    psv = ps_x.tile([128, C], F32, tag="x", name=f"v_{pt}")
                    for ct in range(CT):
                        nc.tensor.matmul(
                            psv[:],
                            xo[ct][:, 128 * pt:128 * (pt + 1)],
                            wqkv_r[ct][:, 2 * C:3 * C],
                            start=(ct == 0), stop=(ct == CT - 1),
                        )
                    nc.vector.tensor_copy(v_r[:, C * pt:C * (pt + 1)], psv[:])

                # ---- attention, software-pipelined over 32 k-tiles ----
                def scores(qs, kt, qc):
                    ss = ps_s.tile([128, 512], F32, tag="s", name=f"ss_{qc}_{kt}")
                    for ct in range(CT):
                        nc.tensor.matmul(
                            ss[:],
                            k_r[ct][:, 128 * kt:128 * (kt + 1)],
                            qt_r[ct][:, qs],
                            start=(ct == 0), stop=(ct == CT - 1),
                        )
                    ex = expp.tile([128, 512], ATTN_DT, tag="ex", name=f"ex_{qc}_{kt}")
                    nc.scalar.activation(ex[:], ss[:], AF.Exp, scale=SCALE)
                    return ex

                for qc in range(n_qc):
                    qs = slice(512 * qc, 512 * (qc + 1))
                    av = [ps_av.tile([128, 512], F32, tag="av", name=f"av_{qc}_{i}")
                          for i in range(CT)]
                    # softmax denominator accumulates on DVE (frees PE time)
                    acc = dnp.tile([128, 512], F32, tag="dnacc", name=f"dnacc_{qc}")

                    def av_step(ex, kt, av=av):
                        for ct in range(CT):
                            nc.tensor.matmul(
                                av[ct][:],
                                v_r[:, C * kt + 128 * ct:C * kt + 128 * (ct + 1)],
                                ex[:],
                                start=(kt == 0), stop=(kt == 31),
                            )

                    prev_ex = scores(qs, 0, qc)
                    for kt in range(1, 32):
                        ex = scores(qs, kt, qc)
                        av_step(prev_ex, kt - 1)
                        if kt == 1:
                            nc.vector.tensor_tensor(
                                out=acc[:], in0=prev_ex[:], in1=ex[:], op=ALU.add)
                        else:
                            nc.vector.tensor_add(acc[:], acc[:], ex[:])
                        prev_ex = ex
                    av_step(prev_ex, 31)
                    # partition reduction of the accumulated exp sums
                    dnps = ps_x.tile([1, 512], F32, tag="x", name=f"dnps_{qc}")
                    nc.tensor.matmul(dnps[:], ones_col_f[:], acc[:], start=True, stop=True)

                    # normalize: attn^T[c,q] = av * (1/dn) + bv'
                    rec = vec.tile([1, 512], F32, tag="rec", name=f"rec_{qc}")
                    nc.vector.reciprocal(rec[:], dnps[:])
                    rec_r = vec.tile([1, 512], F32R, tag="rec_r", name=f"recr_{qc}")
                    nc.vector.tensor_copy(rec_r[:], rec[:])
                    bc = ps_x.tile([128, 512], F32, tag="x", name=f"bc_{qc}")
                    nc.tensor.matmul(bc[:], ones_row_r[:], rec_r[:], start=True, stop=True)
                    bc_sb = attnp.tile([128, 512], F32, tag="bc_sb", name=f"bcsb_{qc}")
                    nc.scalar.mul(bc_sb[:], bc[:], 1.0)
                    at_sb = []
                    for ct in range(CT):
                        at = attnp.tile([128, 512], MM_DT, tag=f"at_{ct}", name=f"at_{qc}_{ct}")
                        nc.vector.tensor_tensor(out=at[:], in0=av[ct][:], in1=bc_sb[:], op=ALU.mult)
                        nc.vector.tensor_scalar_add(at[:], at[:], bq_fold[4 + ct][:])
                        at_sb.append(at)
                    # output projection + bias + residual
                    for ot in range(CT):
                        po = ps_x.tile([128, 512], F32, tag="x", name=f"po_{qc}_{ot}")
                        for ct in range(CT):
                            nc.tensor.matmul(
                                po[:],
                                wout_r[ct][:, 128 * ot:128 * (ot + 1)],
                                at_sb[ct][:],
                                start=(ct == 0), stop=(ct == CT - 1),
                            )
                        fin = outp.tile([128, 512], F32, tag="fin", name=f"fin_{qc}_{ot}")
                        nc.vector.tensor_scalar_add(fin[:], po[:], bo_col[ot][:])
                        nc.vector.tensor_tensor(out=fin[:], in0=fin[:], in1=xq[ot][:, qs], op=ALU.add)
                        nc.sync.dma_start(out_d[128 * ot:128 * (ot + 1), qs], fin[:])

    nc.finalize()
    return nc


_NC_CACHE = None


def _get_nc(n_reps: int = 1):
    global _NC_CACHE
    if _NC_CACHE is None:
        _NC_CACHE = _build(n_reps)
    return _NC_CACHE


def make_in_maps(inputs):
    """Per-core input staging: layout/dtype only (no math)."""
    x = np.asarray(inputs["x"], np.float32)
    W_qkv = np.asarray(inputs["W_qkv"], np.float32)
    W_out = np.asarray(inputs["W_out"], np.float32)
    bf16 = mybir.dt.np(BF16)

    w_qkv_t = np.ascontiguousarray(W_qkv.T)          # [256, 768]
    w_out_t = np.ascontiguousarray(W_out.T)          # [256, 256]
    bq2 = np.asarray(inputs["b_qkv"], np.float32).reshape(3 * C, 1)
    bo2 = np.asarray(inputs["b_out"], np.float32).reshape(C, 1)
    ga2 = np.asarray(inputs["gamma"], np.float32).reshape(C, 1)
    be2 = np.asarray(inputs["beta"], np.float32).reshape(C, 1)

    xf = x.reshape(B, C, NPOS)
    in_maps = []
    for core in range(N_CORES):
        item, half = divmod(core, 2)
        xi = xf[item]
        if half == 0:
            own = xi
        else:
            own = np.concatenate([xi[:, NQ:], xi[:, :NQ]], axis=1)
        others = [xf[j] for j in range(B) if j != item]
        xb_np = np.ascontiguousarray(
            np.concatenate([own] + others, axis=1)).astype(bf16)
        in_maps.append({
            "xb": xb_np,
            "xq32": np.ascontiguousarray(own[:, :NQ]),
            "w_qkv_t": w_qkv_t,
            "w_out_t": w_out_t,
            "b_qkv": bq2,
            "b_out": bo2,
            "gamma": ga2,
            "beta": be2,
        })
    return in_maps


def kernel(x, W_qkv, b_qkv, W_out, b_out, gamma, beta):
    nc = _get_nc()
    in_maps = make_in_maps({
        "x": x, "W_qkv": W_qkv, "b_qkv": b_qkv, "W_out": W_out,
        "b_out": b_out, "gamma": gamma, "beta": beta,
    })
    res = bass_utils.run_bass_kernel_spmd(nc, in_maps, core_ids=list(range(N_CORES)))

    out = np.empty((B, C, NPOS), dtype=np.float32)
    for core in range(N_CORES):
        item, half = divmod(core, 2)
        out[item][:, NQ * half:NQ * (half + 1)] = res.results[core]["out"]
    return out.reshape(B, C, H, W)
